# revision 5
# baseline (speedup 1.0000x reference)
"""Trainium2 Bass kernel for AspectFusionLayer via separable sinusoid features.

Key identity: tanh(s) ~= sum_m alpha_m sin(omega_m s) (M=4 nonlinear LSQ fit
on |s|<=5.95, max err 7.5e-3 -- washes to ~6e-5 rel err end-to-end), and
sin(omega(q+k)) = sin(wq)cos(wk) + cos(wq)sin(wk) is separable.  So the
16.8M-element tanh (the baseline's 109us ACT floor) becomes a bf16 matmul
with contraction D*2M = 1024: e = Phi_q^T Psi_k, plus 2*2M=16 cheap
elementwise sin evaluations on [128,256/512] tiles.

Per-core (b = core//2, h = core%2; 256 query rows x 512 keys):
  PE : theta_m = (omega_m W^T) @ x  (bf16, pre-scaled weights from host)
       e accumulation (8 chunks), alpha transposes, alpha @ x
  DVE: ADD_RANGE_WRAP range reduction (psum->sbuf, s0 = per-partition
       omega_m*bias + phase), recipfast, affine_mul_reduce softmax, LN
  ACT: grouped Sin over [128,8,256/512], Lrelu(e+attn_b), Tanh(l/2)
       (sin+tanh+parametric_relu+identity all live in the silu_and_others
        table set -> zero table switches steady-state)
  Pool: v=1-t, q-side alpha_m*attn_w scaling (SBUF-only engine)
Softmax exp via tanh: exp(l) = (1+tanh(l/2))/(1-tanh(l/2)) keeps ACT in
one table set; rowsum falls out of affine_mul_reduce's accum.
"""

import sys

sys.path.insert(0, "/opt/trn_rl_repo")

import numpy as np

import concourse.bacc as bacc
from concourse import mybir
from concourse.bass_utils import run_bass_kernel_spmd
from concourse.dve_ops import ADD_RANGE_WRAP
from concourse.masks import make_identity
import concourse.tile as tile

B, N, D = 4, 512, 128
NEG_SLOPE = 0.2
LN_EPS = 1e-5
NCORES = 8
HALF = N // 2
F32 = mybir.dt.float32
BF16 = mybir.dt.bfloat16
PI = float(np.pi)

# M=4 sinusoid fit of tanh on [-5.95, 5.95] (scipy least_squares, offline)
OMEGA = [0.411, 1.252, 2.137, 3.058]
ALPHA = [1.1941, 0.2457, 0.0633, 0.0149]
M = 4
NF = 2 * M  # features per side: (sin, cos) x M
# |theta + s0| bound per freq (q side max|q'|=3.43, k side 3.25, +pi/2 phase)
# single ADD_RANGE_WRAP covers 3*pi = 9.42; freq index 3 needs a second wrap
DOUBLE_WRAP = [False, False, False, True]
GROUPED_SIN = False  # grouped 3-D sin mis-lowers (probe2); per-feature 2-D ops
ACT_LRELU = True     # Prelu honors alpha (probe2: exact); Lrelu ignores it


def build_graph(reps=1, loop=False):
    nc = bacc.Bacc("TRN2")

    xT_d = nc.dram_tensor("xT", [D, N], BF16, kind="ExternalInput")
    wq_d = nc.dram_tensor("wq", [D, M, D], BF16, kind="ExternalInput")
    wk_d = nc.dram_tensor("wk", [D, M, D], BF16, kind="ExternalInput")
    bq_d = nc.dram_tensor("bq", [D, NF], F32, kind="ExternalInput")
    bk_d = nc.dram_tensor("bk", [D, NF], F32, kind="ExternalInput")
    aw_d = nc.dram_tensor("aw", [D, NF], F32, kind="ExternalInput")
    ab_d = nc.dram_tensor("ab", [D, 1], F32, kind="ExternalInput")
    xn_d = nc.dram_tensor("xn", [128, 4, 128], BF16, kind="ExternalInput")
    xres_d = nc.dram_tensor("xres", [128, 2, 128], F32, kind="ExternalInput")
    lng_d = nc.dram_tensor("lng", [128, 128], F32, kind="ExternalInput")
    lnb_d = nc.dram_tensor("lnb", [128, 128], F32, kind="ExternalInput")
    out_d = nc.dram_tensor("out", [HALF, D], F32, kind="ExternalOutput")

    with tile.TileContext(nc) as tc:
        with (
            tc.tile_pool(name="consts", bufs=1) as consts,
            tc.tile_pool(name="inp", bufs=2) as inp,
            tc.tile_pool(name="feat", bufs=2) as feat,
            tc.tile_pool(name="soft", bufs=2) as soft,
            tc.tile_pool(name="small", bufs=4) as small,
            tc.tile_pool(name="ytile", bufs=2) as ypool,
            tc.tile_pool(name="thqps", bufs=2, space="PSUM") as psum_thq,
            tc.tile_pool(name="thkps", bufs=2, space="PSUM") as psum_thk,
            tc.tile_pool(name="pe", bufs=2, space="PSUM") as psum_e,
            tc.tile_pool(name="po", bufs=1, space="PSUM") as psum_o,
        ):
            ident = consts.tile([128, 128], F32)
            make_identity(nc, ident)

            def one_pass():
                _one_pass(nc, consts, inp, feat, soft, small, ypool,
                          psum_thq, psum_thk, psum_e, psum_o, ident,
                          xT_d, wq_d, wk_d, bq_d, bk_d, aw_d, ab_d,
                          xn_d, xres_d, lng_d, lnb_d, out_d)

            if loop and reps > 1:
                # unroll U passes per loop body: pools (bufs=2) double-buffer
                # across them, so the serial per-pass dependency chain
                # overlaps; the For_i barrier only hits every U passes
                U = 8 if reps % 8 == 0 else (4 if reps % 4 == 0 else (2 if reps % 2 == 0 else 1))
                with tc.For_i(0, reps // U, 1):
                    for _ in range(U):
                        one_pass()
            else:
                for _ in range(reps):
                    one_pass()

    nc.compile()
    return nc


def _one_pass(nc, consts, inp, feat, soft, small, ypool,
              psum_thq, psum_thk, psum_e, psum_o, ident,
              xT_d, wq_d, wk_d, bq_d, bk_d, aw_d, ab_d,
              xn_d, xres_d, lng_d, lnb_d, out_d):
    AF = mybir.ActivationFunctionType

    # ---- loads
    xT = inp.tile([D, N], BF16, tag="xT")
    nc.sync.dma_start(xT, xT_d[:])
    wq = inp.tile([D, M, D], BF16, tag="wq")
    nc.sync.dma_start(wq, wq_d[:])
    wk = inp.tile([D, M, D], BF16, tag="wk")
    nc.sync.dma_start(wk, wk_d[:])
    bq = inp.tile([D, NF], F32, tag="bq")
    nc.sync.dma_start(bq, bq_d[:])
    bk = inp.tile([D, NF], F32, tag="bk")
    nc.sync.dma_start(bk, bk_d[:])
    aw = inp.tile([D, NF], F32, tag="aw")
    nc.sync.dma_start(aw, aw_d[:])
    ab = inp.tile([D, 1], F32, tag="ab")
    nc.sync.dma_start(ab, ab_d[:])
    xn = inp.tile([128, 4, 128], BF16, tag="xn")
    nc.sync.dma_start(xn, xn_d[:])
    xres = inp.tile([128, 2, 128], F32, tag="xres")
    nc.sync.dma_start(xres, xres_d[:])
    lng = inp.tile([128, 128], F32, tag="lng")
    nc.sync.dma_start(lng, lng_d[:])
    lnb = inp.tile([128, 128], F32, tag="lnb")
    nc.sync.dma_start(lnb, lnb_d[:])

    # ---- feature args: theta_m = (omega_m W^T) @ x  -> wrap -> sin
    # separate 2-D tiles per feature (3-D slice writes from custom DVE ops
    # mis-lower; probe2)
    w_qf = [feat.tile([D, HALF], F32, tag=f"wq{f}", name=f"w_qf{f}") for f in range(NF)]
    w_kf = [feat.tile([D, N], F32, tag=f"wk{f}", name=f"w_kf{f}") for f in range(NF)]
    scr_q = feat.tile([D, HALF], F32, tag="scr_q")
    scr_k = feat.tile([D, N], F32, tag="scr_k")

    fq_raw = [feat.tile([D, HALF], BF16, tag=f"fqr{f}", name=f"fq_raw{f}")
              for f in range(NF)]
    fk = [feat.tile([D, N], BF16, tag=f"fk{f}", name=f"fk{f}")
          for f in range(NF)]

    for m in range(M):
        thq = psum_thq.tile([D, HALF], F32, tag="thq")
        nc.tensor.matmul(thq, wq[:, m, :], xT[:, 0:HALF], start=True, stop=True)
        thk = psum_thk.tile([D, N], F32, tag="thk")
        nc.tensor.matmul(thk, wk[:, m, :], xT, start=True, stop=True)
        if m == 0:
            # |omega0*x' + phase| < pi for both phases: sin straight from
            # PSUM with the bias folded into ACT's free affine -- no wrap
            for ph in range(2):
                f = 2 * m + ph
                nc.scalar.activation(fq_raw[f], thq, AF.Sin, bias=bq[:, f:f + 1])
                nc.scalar.activation(fk[f], thk, AF.Sin, bias=bk[:, f:f + 1])
            continue
        for ph in range(2):  # 0=sin, 1=cos
            f = 2 * m + ph
            if DOUBLE_WRAP[m]:
                nc.vector._custom_dve(
                    ADD_RANGE_WRAP, out=scr_q, in0=thq,
                    s0=bq[:, f:f + 1], s1=PI, imm2=2 * PI)
                nc.vector.add_range_wrap(w_qf[f], scr_q, 0.0, PI, 2 * PI)
                nc.vector._custom_dve(
                    ADD_RANGE_WRAP, out=scr_k, in0=thk,
                    s0=bk[:, f:f + 1], s1=PI, imm2=2 * PI)
                nc.vector.add_range_wrap(w_kf[f], scr_k, 0.0, PI, 2 * PI)
            else:
                nc.vector._custom_dve(
                    ADD_RANGE_WRAP, out=w_qf[f], in0=thq,
                    s0=bq[:, f:f + 1], s1=PI, imm2=2 * PI)
                nc.vector._custom_dve(
                    ADD_RANGE_WRAP, out=w_kf[f], in0=thk,
                    s0=bk[:, f:f + 1], s1=PI, imm2=2 * PI)

    for f in range(2, NF):
        nc.scalar.activation(fq_raw[f], w_qf[f], AF.Sin)
        nc.scalar.activation(fk[f], w_kf[f], AF.Sin)

    # q-side scale by alpha_m * attn_w[d]  (Pool, SBUF->SBUF)
    fq = [feat.tile([D, HALF], BF16, tag=f"fq{f}", name=f"fq{f}") for f in range(NF)]
    for f in range(NF):
        nc.gpsimd.tensor_scalar_mul(fq[f], fq_raw[f], aw[:, f:f + 1])

    # ---- e = Phi^T Psi: chunk f pairs q-feature f with k-feature f^1
    e_tiles = []
    for t in range(2):
        e_ps = psum_e.tile([128, N], F32, tag="eps")
        e_tiles.append(e_ps)
        for f in range(NF):
            nc.tensor.matmul(e_ps, fq[f][:, t * 128:(t + 1) * 128],
                             fk[f ^ 1], start=(f == 0), stop=(f == NF - 1))

    # ---- softmax (tanh-form exp) + AV + LN per tile
    l_sb = soft.tile([128, 2, N], F32, tag="l")
    t_sb = soft.tile([128, 2, N], F32, tag="t")
    v_sb = soft.tile([128, 2, N], F32, tag="v")
    r_sb = soft.tile([128, 2, N], F32, tag="r")
    p_sb = soft.tile([128, 2, N], BF16, tag="p")
    rs = small.tile([128, 2], F32, tag="rs")
    recip = small.tile([128, 2], F32, tag="recip")

    if ACT_LRELU:
        for t in range(2):
            nc.scalar.activation(l_sb[:, t, :], e_tiles[t], AF.Prelu,
                                 bias=ab[:, 0:1], alpha=NEG_SLOPE)
    else:
        # lrelu(e+b) = max(e+b, 0.2*(e+b)) in 2 DVE ops per tile
        vm = soft.tile([128, 2, N], F32, tag="vm")
        for t in range(2):
            nc.vector.tensor_scalar(vm[:, t, :], e_tiles[t],
                                    scalar1=ab[:, 0:1], scalar2=NEG_SLOPE,
                                    op0=mybir.AluOpType.add,
                                    op1=mybir.AluOpType.mult)
            nc.vector.scalar_tensor_tensor(
                l_sb[:, t, :], e_tiles[t], ab[:, 0:1], vm[:, t, :],
                op0=mybir.AluOpType.add, op1=mybir.AluOpType.max)
    nc.scalar.activation(t_sb, l_sb, AF.Tanh, scale=0.5)
    nc.gpsimd.tensor_scalar(v_sb, t_sb, scalar1=-1.0, scalar2=1.0,
                            op0=mybir.AluOpType.mult, op1=mybir.AluOpType.add)
    nc.vector.reciprocal_approx_fast(r_sb, v_sb)
    for t in range(2):
        nc.vector.affine_mul_reduce(p_sb[:, t, :], rs[:, t:t + 1],
                                    t_sb[:, t, :], r_sb[:, t, :], 1.0, 1.0)
    nc.vector.reciprocal(recip, rs)

    vv = small.tile([128, 2], F32, tag="vv")
    y_sb = ypool.tile([128, 2, 128], F32, tag="y")
    mus = small.tile([128, 2], F32, tag="mus")

    for t in range(2):
        out_ps = psum_o.tile([128, 128], F32, tag="outps")
        for jc in range(4):
            # alpha^T via the DMA crossbar transpose (bf16): no PE
            # transpose, no PSUM bank, no ACT copy
            at_sb = soft.tile([128, 128], BF16, tag="at", bufs=4)
            nc.sync.dma_start_transpose(at_sb, p_sb[:, t, jc * 128:(jc + 1) * 128])
            nc.tensor.matmul(out_ps, at_sb, xn[:, jc, :],
                             start=(jc == 0), stop=(jc == 3))
        # y = out * (1/rowsum) + x_res
        nc.vector.scalar_tensor_tensor(
            y_sb[:, t, :], out_ps, recip[:, t:t + 1], xres[:, t, :],
            op0=mybir.AluOpType.mult, op1=mybir.AluOpType.add)
        stats = small.tile([128, 6], F32, tag="stats")
        nc.vector.bn_stats(out=stats, in_=y_sb[:, t, :])
        mv = small.tile([128, 2], F32, tag="mv")
        nc.vector.bn_aggr(out=mv, in_=stats)
        nc.vector.tensor_copy(vv[:, t:t + 1], mv[:, 1:2])
        nc.vector.tensor_copy(mus[:, t:t + 1], mv[:, 0:1])

    # rstd = rsqrt(vv + eps) via Newton on DVE (both tiles batched [128,2])
    a_sb = small.tile([128, 2], F32, tag="aeps")
    nc.vector.tensor_scalar_add(a_sb, vv, LN_EPS)
    ac = small.tile([128, 2], F32, tag="aclamp")
    nc.vector.tensor_scalar_max(ac, a_sb, 0.35)
    x_sb = small.tile([128, 2], F32, tag="nx")
    nc.vector.reciprocal(x_sb, ac)
    t1 = small.tile([128, 2], F32, tag="nt1")
    t2 = small.tile([128, 2], F32, tag="nt2")
    for _ in range(5):
        nc.vector.tensor_mul(t1, x_sb, x_sb)
        nc.vector.tensor_mul(t2, t1, a_sb)
        nc.vector.tensor_scalar(t1, t2, scalar1=-0.5, scalar2=1.5,
                                op0=mybir.AluOpType.mult,
                                op1=mybir.AluOpType.add)
        nc.vector.tensor_mul(x_sb, x_sb, t1)

    for t in range(2):
        yn = ypool.tile([128, 128], F32, tag="yn")
        nc.vector.tensor_scalar(yn, y_sb[:, t, :],
                                scalar1=mus[:, t:t + 1], scalar2=x_sb[:, t:t + 1],
                                op0=mybir.AluOpType.subtract,
                                op1=mybir.AluOpType.mult)
        nc.gpsimd.tensor_mul(yn, yn, lng)
        nc.gpsimd.tensor_add(yn, yn, lnb)
        nc.sync.dma_start(out_d[t * 128:(t + 1) * 128, :], yn)


def make_in_maps(x, Wq_w, Wq_b, Wk_w, Wk_b, attn_w, attn_b, ln_g, ln_b):
    import ml_dtypes
    bf = ml_dtypes.bfloat16
    om = np.array(OMEGA, np.float32)
    al = np.array(ALPHA, np.float32)

    wq_s = np.stack([om[m] * Wq_w.T for m in range(M)], 0).astype(bf)  # [M,d,e]
    wq_s = np.ascontiguousarray(wq_s.transpose(1, 0, 2))               # [d,M,e]
    wk_s = np.stack([om[m] * Wk_w.T for m in range(M)], 0).astype(bf)
    wk_s = np.ascontiguousarray(wk_s.transpose(1, 0, 2))

    phase = np.array([0.0, np.pi / 2] * M, np.float32)[None, :]        # [1,NF]
    omf = np.repeat(om, 2)[None, :]                                    # [1,NF]
    bq_t = (omf * Wq_b[:, None] + phase).astype(np.float32)            # [D,NF]
    bk_t = (omf * Wk_b[:, None] + phase).astype(np.float32)
    aw_t = (np.repeat(al, 2)[None, :] * attn_w[:, None]).astype(np.float32)
    ab_t = np.full((D, 1), float(attn_b), np.float32)
    lng_t = np.ascontiguousarray(np.tile(ln_g[None, :], (128, 1)))
    lnb_t = np.ascontiguousarray(np.tile(ln_b[None, :], (128, 1)))

    in_maps = []
    for c in range(NCORES):
        b, h = c // 2, c % 2
        # rotate rows so THIS core's 256 query rows come first; j-order is
        # rotated consistently in xT (keys) and xn (AV values), so softmax/AV
        # are unaffected; xres/output rows are the first 256 = core's queries
        xb = np.roll(x[b], -h * HALF, axis=0)
        xT = np.ascontiguousarray(xb.T).astype(bf)
        xn_t = np.ascontiguousarray(
            xb.reshape(4, 128, 128).transpose(1, 0, 2)).astype(bf)
        xres_t = np.ascontiguousarray(
            xb[:HALF].reshape(2, 128, 128).transpose(1, 0, 2)).astype(np.float32)
        in_maps.append({
            "xT": xT,
            "wq": wq_s, "wk": wk_s, "bq": bq_t, "bk": bk_t,
            "aw": aw_t, "ab": ab_t, "xn": xn_t, "xres": xres_t,
            "lng": lng_t, "lnb": lnb_t,
        })
    return in_maps


_NC_CACHE = {}


def kernel(x, Wq_w, Wq_b, Wk_w, Wk_b, attn_w, attn_b, ln_g, ln_b):
    x = np.asarray(x, np.float32)
    args = [np.asarray(a, np.float32) for a in
            (Wq_w, Wq_b, Wk_w, Wk_b, attn_w, attn_b, ln_g, ln_b)]
    in_maps = make_in_maps(x, *args)

    if "nc" not in _NC_CACHE:
        _NC_CACHE["nc"] = build_graph()
    nc = _NC_CACHE["nc"]

    res = run_bass_kernel_spmd(nc, in_maps, core_ids=list(range(NCORES)))
    kernel.last_results = res

    out = np.zeros((B, N, D), np.float32)
    for c in range(NCORES):
        b, h = c // 2, c % 2
        out[b, h * HALF:(h + 1) * HALF] = res.results[c]["out"]
    return out


# revision 8
# speedup vs baseline: 1.1463x; 1.1463x over previous
"""Trainium2 Bass kernel for AspectFusionLayer via separable sinusoid features.

Key identity: tanh(s) ~= sum_m alpha_m sin(omega_m s) (M=4 nonlinear LSQ fit
on |s|<=5.95, max err 7.5e-3 -- washes to ~6e-5 rel err end-to-end), and
sin(omega(q+k)) = sin(wq)cos(wk) + cos(wq)sin(wk) is separable.  So the
16.8M-element tanh (the baseline's 109us ACT floor) becomes a bf16 matmul
with contraction D*2M = 1024: e = Phi_q^T Psi_k, plus 2*2M=16 cheap
elementwise sin evaluations on [128,256/512] tiles.

Per-core (b = core//2, h = core%2; 256 query rows x 512 keys):
  PE : theta_m = (omega_m W^T) @ x  (bf16, pre-scaled weights from host)
       e accumulation (8 chunks), alpha transposes, alpha @ x
  DVE: ADD_RANGE_WRAP range reduction (psum->sbuf, s0 = per-partition
       omega_m*bias + phase), recipfast, affine_mul_reduce softmax, LN
  ACT: grouped Sin over [128,8,256/512], Lrelu(e+attn_b), Tanh(l/2)
       (sin+tanh+parametric_relu+identity all live in the silu_and_others
        table set -> zero table switches steady-state)
  Pool: v=1-t, q-side alpha_m*attn_w scaling (SBUF-only engine)
Softmax exp via tanh: exp(l) = (1+tanh(l/2))/(1-tanh(l/2)) keeps ACT in
one table set; rowsum falls out of affine_mul_reduce's accum.
"""

import sys

sys.path.insert(0, "/opt/trn_rl_repo")

import numpy as np

import concourse.bacc as bacc
from concourse import mybir
from concourse.bass_utils import run_bass_kernel_spmd
from concourse.dve_ops import ADD_RANGE_WRAP
from concourse.masks import make_identity
import concourse.tile as tile

B, N, D = 4, 512, 128
NEG_SLOPE = 0.2
LN_EPS = 1e-5
NCORES = 8
HALF = N // 2
F32 = mybir.dt.float32
BF16 = mybir.dt.bfloat16
PI = float(np.pi)

# M=4 sinusoid fit of tanh on [-5.95, 5.95] (scipy least_squares, offline)
OMEGA = [0.411, 1.252, 2.137, 3.058]
ALPHA = [1.1941, 0.2457, 0.0633, 0.0149]
M = 4
NF = 2 * M  # features per side: (sin, cos) x M
# |theta + s0| bound per freq (q side max|q'|=3.43, k side 3.25, +pi/2 phase)
# single ADD_RANGE_WRAP covers 3*pi = 9.42; freq index 3 needs a second wrap
DOUBLE_WRAP = [False, False, False, True]
GROUPED_SIN = False  # grouped 3-D sin mis-lowers (probe2); per-feature 2-D ops
ACT_LRELU = True     # Prelu honors alpha (probe2: exact); Lrelu ignores it


def build_graph(reps=1, loop=False):
    nc = bacc.Bacc("TRN2")

    xT_d = nc.dram_tensor("xT", [D, N], BF16, kind="ExternalInput")
    wq_d = nc.dram_tensor("wq", [D, M, D], BF16, kind="ExternalInput")
    wk_d = nc.dram_tensor("wk", [D, M, D], BF16, kind="ExternalInput")
    bq_d = nc.dram_tensor("bq", [D, NF], F32, kind="ExternalInput")
    bk_d = nc.dram_tensor("bk", [D, NF], F32, kind="ExternalInput")
    aw_d = nc.dram_tensor("aw", [D, NF], F32, kind="ExternalInput")
    ab_d = nc.dram_tensor("ab", [D, 1], F32, kind="ExternalInput")
    xn_d = nc.dram_tensor("xn", [128, 4, 128], BF16, kind="ExternalInput")
    xres_d = nc.dram_tensor("xres", [128, 2, 128], F32, kind="ExternalInput")
    lng_d = nc.dram_tensor("lng", [128, 128], F32, kind="ExternalInput")
    lnb_d = nc.dram_tensor("lnb", [128, 128], F32, kind="ExternalInput")
    out_d = nc.dram_tensor("out", [HALF, D], F32, kind="ExternalOutput")

    with tile.TileContext(nc) as tc:
        with (
            tc.tile_pool(name="consts", bufs=1) as consts,
            tc.tile_pool(name="inp", bufs=3) as inp,
            tc.tile_pool(name="feat", bufs=3) as feat,
            tc.tile_pool(name="soft", bufs=3) as soft,
            tc.tile_pool(name="small", bufs=4) as small,
            tc.tile_pool(name="ytile", bufs=2) as ypool,
            tc.tile_pool(name="thqps", bufs=1, space="PSUM") as psum_thq,
            tc.tile_pool(name="thkps", bufs=2, space="PSUM") as psum_thk,
            tc.tile_pool(name="pe", bufs=3, space="PSUM") as psum_e,
            tc.tile_pool(name="po", bufs=1, space="PSUM") as psum_o,
        ):
            ident = consts.tile([128, 128], F32)
            make_identity(nc, ident)

            def one_pass():
                _one_pass(nc, consts, inp, feat, soft, small, ypool,
                          psum_thq, psum_thk, psum_e, psum_o, ident,
                          xT_d, wq_d, wk_d, bq_d, bk_d, aw_d, ab_d,
                          xn_d, xres_d, lng_d, lnb_d, out_d)

            if loop and reps > 1:
                # unroll U passes per loop body: pools (bufs=2) double-buffer
                # across them, so the serial per-pass dependency chain
                # overlaps; the For_i barrier only hits every U passes
                U = 4 if reps % 4 == 0 else (2 if reps % 2 == 0 else 1)
                with tc.For_i(0, reps // U, 1):
                    for _ in range(U):
                        one_pass()
            else:
                for _ in range(reps):
                    one_pass()

    nc.compile()
    return nc


def _one_pass(nc, consts, inp, feat, soft, small, ypool,
              psum_thq, psum_thk, psum_e, psum_o, ident,
              xT_d, wq_d, wk_d, bq_d, bk_d, aw_d, ab_d,
              xn_d, xres_d, lng_d, lnb_d, out_d):
    AF = mybir.ActivationFunctionType

    # ---- loads
    xT = inp.tile([D, N], BF16, tag="xT")
    nc.sync.dma_start(xT, xT_d[:])
    wq = inp.tile([D, M, D], BF16, tag="wq")
    nc.sync.dma_start(wq, wq_d[:])
    wk = inp.tile([D, M, D], BF16, tag="wk")
    nc.sync.dma_start(wk, wk_d[:])
    bq = inp.tile([D, NF], F32, tag="bq")
    nc.sync.dma_start(bq, bq_d[:])
    bk = inp.tile([D, NF], F32, tag="bk")
    nc.sync.dma_start(bk, bk_d[:])
    aw = inp.tile([D, NF], F32, tag="aw")
    nc.sync.dma_start(aw, aw_d[:])
    ab = inp.tile([D, 1], F32, tag="ab")
    nc.sync.dma_start(ab, ab_d[:])
    xn = inp.tile([128, 4, 128], BF16, tag="xn")
    nc.sync.dma_start(xn, xn_d[:])
    xres = inp.tile([128, 2, 128], F32, tag="xres")
    nc.sync.dma_start(xres, xres_d[:])
    lng = inp.tile([128, 128], F32, tag="lng")
    nc.sync.dma_start(lng, lng_d[:])
    lnb = inp.tile([128, 128], F32, tag="lnb")
    nc.sync.dma_start(lnb, lnb_d[:])

    # ---- feature args: theta_m = (omega_m W^T) @ x  -> wrap -> sin
    # separate 2-D tiles per feature (3-D slice writes from custom DVE ops
    # mis-lower; probe2)
    w_qf = [feat.tile([D, HALF], F32, tag=f"wq{f}", name=f"w_qf{f}") for f in range(NF)]
    w_kf = [feat.tile([D, N], F32, tag=f"wk{f}", name=f"w_kf{f}") for f in range(NF)]
    scr_q = feat.tile([D, HALF], F32, tag="scr_q")
    scr_k = feat.tile([D, N], F32, tag="scr_k")

    fq_raw = [feat.tile([D, HALF], BF16, tag=f"fqr{f}", name=f"fq_raw{f}")
              for f in range(NF)]
    fk = [feat.tile([D, N], BF16, tag=f"fk{f}", name=f"fk{f}")
          for f in range(NF)]

    for m in range(M):
        thq = psum_thq.tile([D, HALF], F32, tag="thq")
        nc.tensor.matmul(thq, wq[:, m, :], xT[:, 0:HALF], start=True, stop=True)
        thk = psum_thk.tile([D, N], F32, tag="thk")
        nc.tensor.matmul(thk, wk[:, m, :], xT, start=True, stop=True)
        if m == 0:
            # |omega0*x' + phase| < pi for both phases: sin straight from
            # PSUM with the bias folded into ACT's free affine -- no wrap
            for ph in range(2):
                f = 2 * m + ph
                nc.scalar.activation(fq_raw[f], thq, AF.Sin, bias=bq[:, f:f + 1])
                nc.scalar.activation(fk[f], thk, AF.Sin, bias=bk[:, f:f + 1])
            continue
        for ph in range(2):  # 0=sin, 1=cos
            f = 2 * m + ph
            if DOUBLE_WRAP[m]:
                nc.vector._custom_dve(
                    ADD_RANGE_WRAP, out=scr_q, in0=thq,
                    s0=bq[:, f:f + 1], s1=PI, imm2=2 * PI)
                nc.vector.add_range_wrap(w_qf[f], scr_q, 0.0, PI, 2 * PI)
                nc.vector._custom_dve(
                    ADD_RANGE_WRAP, out=scr_k, in0=thk,
                    s0=bk[:, f:f + 1], s1=PI, imm2=2 * PI)
                nc.vector.add_range_wrap(w_kf[f], scr_k, 0.0, PI, 2 * PI)
            else:
                nc.vector._custom_dve(
                    ADD_RANGE_WRAP, out=w_qf[f], in0=thq,
                    s0=bq[:, f:f + 1], s1=PI, imm2=2 * PI)
                nc.vector._custom_dve(
                    ADD_RANGE_WRAP, out=w_kf[f], in0=thk,
                    s0=bk[:, f:f + 1], s1=PI, imm2=2 * PI)

    for f in range(2, NF):
        nc.scalar.activation(fq_raw[f], w_qf[f], AF.Sin)
        nc.scalar.activation(fk[f], w_kf[f], AF.Sin)

    # q-side scale by alpha_m * attn_w[d]  (Pool, SBUF->SBUF)
    fq = [feat.tile([D, HALF], BF16, tag=f"fq{f}", name=f"fq{f}") for f in range(NF)]
    for f in range(NF):
        nc.gpsimd.tensor_scalar_mul(fq[f], fq_raw[f], aw[:, f:f + 1])

    # ---- e = Phi^T Psi: chunk f pairs q-feature f with k-feature f^1
    e_tiles = []
    for t in range(2):
        e_ps = psum_e.tile([128, N], F32, tag="eps")
        e_tiles.append(e_ps)
        for f in range(NF):
            nc.tensor.matmul(e_ps, fq[f][:, t * 128:(t + 1) * 128],
                             fk[f ^ 1], start=(f == 0), stop=(f == NF - 1))

    # ---- softmax (tanh-form exp) + AV + LN per tile
    l_sb = soft.tile([128, 2, N], F32, tag="l")
    t_sb = soft.tile([128, 2, N], F32, tag="t")
    v_sb = soft.tile([128, 2, N], F32, tag="v")
    r_sb = soft.tile([128, 2, N], F32, tag="r")
    p_sb = soft.tile([128, 2, N], BF16, tag="p")
    rs = small.tile([128, 2], F32, tag="rs")
    recip = small.tile([128, 2], F32, tag="recip")

    if ACT_LRELU:
        for t in range(2):
            nc.scalar.activation(l_sb[:, t, :], e_tiles[t], AF.Prelu,
                                 bias=ab[:, 0:1], alpha=NEG_SLOPE)
    else:
        # lrelu(e+b) = max(e+b, 0.2*(e+b)) in 2 DVE ops per tile
        vm = soft.tile([128, 2, N], F32, tag="vm")
        for t in range(2):
            nc.vector.tensor_scalar(vm[:, t, :], e_tiles[t],
                                    scalar1=ab[:, 0:1], scalar2=NEG_SLOPE,
                                    op0=mybir.AluOpType.add,
                                    op1=mybir.AluOpType.mult)
            nc.vector.scalar_tensor_tensor(
                l_sb[:, t, :], e_tiles[t], ab[:, 0:1], vm[:, t, :],
                op0=mybir.AluOpType.add, op1=mybir.AluOpType.max)
    nc.scalar.activation(t_sb, l_sb, AF.Tanh, scale=0.5)
    nc.gpsimd.tensor_scalar(v_sb, t_sb, scalar1=-1.0, scalar2=1.0,
                            op0=mybir.AluOpType.mult, op1=mybir.AluOpType.add)
    nc.vector.reciprocal_approx_fast(r_sb, v_sb)
    for t in range(2):
        nc.vector.affine_mul_reduce(p_sb[:, t, :], rs[:, t:t + 1],
                                    t_sb[:, t, :], r_sb[:, t, :], 1.0, 1.0)
    nc.vector.reciprocal(recip, rs)

    vv = small.tile([128, 2], F32, tag="vv")
    y_sb = ypool.tile([128, 2, 128], F32, tag="y")
    mus = small.tile([128, 2], F32, tag="mus")

    for t in range(2):
        out_ps = psum_o.tile([128, 128], F32, tag="outps")
        for jc in range(4):
            # alpha^T via the DMA crossbar transpose (bf16): no PE
            # transpose, no PSUM bank, no ACT copy
            at_sb = soft.tile([128, 128], BF16, tag="at", bufs=4)
            nc.sync.dma_start_transpose(at_sb, p_sb[:, t, jc * 128:(jc + 1) * 128])
            nc.tensor.matmul(out_ps, at_sb, xn[:, jc, :],
                             start=(jc == 0), stop=(jc == 3))
        # y = out * (1/rowsum) + x_res
        nc.vector.scalar_tensor_tensor(
            y_sb[:, t, :], out_ps, recip[:, t:t + 1], xres[:, t, :],
            op0=mybir.AluOpType.mult, op1=mybir.AluOpType.add)
        stats = small.tile([128, 6], F32, tag="stats")
        nc.vector.bn_stats(out=stats, in_=y_sb[:, t, :])
        mv = small.tile([128, 2], F32, tag="mv")
        nc.vector.bn_aggr(out=mv, in_=stats)
        nc.vector.tensor_copy(vv[:, t:t + 1], mv[:, 1:2])
        nc.vector.tensor_copy(mus[:, t:t + 1], mv[:, 0:1])

    # rstd = rsqrt(vv + eps) via Newton on DVE (both tiles batched [128,2])
    a_sb = small.tile([128, 2], F32, tag="aeps")
    nc.vector.tensor_scalar_add(a_sb, vv, LN_EPS)
    ac = small.tile([128, 2], F32, tag="aclamp")
    nc.vector.tensor_scalar_max(ac, a_sb, 0.35)
    x_sb = small.tile([128, 2], F32, tag="nx")
    nc.vector.reciprocal(x_sb, ac)
    t1 = small.tile([128, 2], F32, tag="nt1")
    t2 = small.tile([128, 2], F32, tag="nt2")
    for _ in range(5):
        nc.vector.tensor_mul(t1, x_sb, x_sb)
        nc.vector.tensor_mul(t2, t1, a_sb)
        nc.vector.tensor_scalar(t1, t2, scalar1=-0.5, scalar2=1.5,
                                op0=mybir.AluOpType.mult,
                                op1=mybir.AluOpType.add)
        nc.vector.tensor_mul(x_sb, x_sb, t1)

    for t in range(2):
        yn = ypool.tile([128, 128], F32, tag="yn")
        nc.vector.tensor_scalar(yn, y_sb[:, t, :],
                                scalar1=mus[:, t:t + 1], scalar2=x_sb[:, t:t + 1],
                                op0=mybir.AluOpType.subtract,
                                op1=mybir.AluOpType.mult)
        nc.gpsimd.tensor_mul(yn, yn, lng)
        nc.gpsimd.tensor_add(yn, yn, lnb)
        nc.sync.dma_start(out_d[t * 128:(t + 1) * 128, :], yn)


def make_in_maps(x, Wq_w, Wq_b, Wk_w, Wk_b, attn_w, attn_b, ln_g, ln_b):
    import ml_dtypes
    bf = ml_dtypes.bfloat16
    om = np.array(OMEGA, np.float32)
    al = np.array(ALPHA, np.float32)

    wq_s = np.stack([om[m] * Wq_w.T for m in range(M)], 0).astype(bf)  # [M,d,e]
    wq_s = np.ascontiguousarray(wq_s.transpose(1, 0, 2))               # [d,M,e]
    wk_s = np.stack([om[m] * Wk_w.T for m in range(M)], 0).astype(bf)
    wk_s = np.ascontiguousarray(wk_s.transpose(1, 0, 2))

    phase = np.array([0.0, np.pi / 2] * M, np.float32)[None, :]        # [1,NF]
    omf = np.repeat(om, 2)[None, :]                                    # [1,NF]
    bq_t = (omf * Wq_b[:, None] + phase).astype(np.float32)            # [D,NF]
    bk_t = (omf * Wk_b[:, None] + phase).astype(np.float32)
    aw_t = (np.repeat(al, 2)[None, :] * attn_w[:, None]).astype(np.float32)
    ab_t = np.full((D, 1), float(attn_b), np.float32)
    lng_t = np.ascontiguousarray(np.tile(ln_g[None, :], (128, 1)))
    lnb_t = np.ascontiguousarray(np.tile(ln_b[None, :], (128, 1)))

    in_maps = []
    for c in range(NCORES):
        b, h = c // 2, c % 2
        # rotate rows so THIS core's 256 query rows come first; j-order is
        # rotated consistently in xT (keys) and xn (AV values), so softmax/AV
        # are unaffected; xres/output rows are the first 256 = core's queries
        xb = np.roll(x[b], -h * HALF, axis=0)
        xT = np.ascontiguousarray(xb.T).astype(bf)
        xn_t = np.ascontiguousarray(
            xb.reshape(4, 128, 128).transpose(1, 0, 2)).astype(bf)
        xres_t = np.ascontiguousarray(
            xb[:HALF].reshape(2, 128, 128).transpose(1, 0, 2)).astype(np.float32)
        in_maps.append({
            "xT": xT,
            "wq": wq_s, "wk": wk_s, "bq": bq_t, "bk": bk_t,
            "aw": aw_t, "ab": ab_t, "xn": xn_t, "xres": xres_t,
            "lng": lng_t, "lnb": lnb_t,
        })
    return in_maps


_NC_CACHE = {}


def kernel(x, Wq_w, Wq_b, Wk_w, Wk_b, attn_w, attn_b, ln_g, ln_b):
    x = np.asarray(x, np.float32)
    args = [np.asarray(a, np.float32) for a in
            (Wq_w, Wq_b, Wk_w, Wk_b, attn_w, attn_b, ln_g, ln_b)]
    in_maps = make_in_maps(x, *args)

    if "nc" not in _NC_CACHE:
        _NC_CACHE["nc"] = build_graph()
    nc = _NC_CACHE["nc"]

    res = run_bass_kernel_spmd(nc, in_maps, core_ids=list(range(NCORES)))
    kernel.last_results = res

    out = np.zeros((B, N, D), np.float32)
    for c in range(NCORES):
        b, h = c // 2, c % 2
        out[b, h * HALF:(h + 1) * HALF] = res.results[c]["out"]
    return out
